# revision 5
# baseline (speedup 1.0000x reference)
"""Trainium2 Bass kernel for nn_Attention_73031623901249.

Multi-head attention with per-head 512x512 projections, interleaved RoPE,
causal softmax, a transposed P^T @ V contraction, and an output projection.

Sharding: one head per NeuronCore (H == 8 == n_cores). Each core computes its
head's full attention plus its slice of the W_o projection; the host sums the
8 partial outputs.

Layout/precision choices:
  - Everything on SBUF is fp16 (inputs are cast host-side): matmul moving
    operands run at 1 col/cycle at any width, and DVE elementwise ops hit
    the 2x fast path (all-SBUF, 2-byte, packed). PSUM stays fp32.
  - q is fed transposed as qT [D, B*S]; W_q / W_k columns are permuted
    even/odd -> [evens | odds] (W_q pre-scaled by 1/sqrt(D)) so interleaved
    RoPE becomes elementwise ops on partition-aligned halves.
  - The kernel is software-pipelined: score row-tiles for chunk j-1 are
    interleaved (at emission granularity) with projections+RoPE of chunk j,
    and the final score wave overlaps the start of the P^T V phase. Scores
    stream through 512-wide PSUM chunks at exact causal width, exp'd (ACT)
    straight to fp16 P tiles.
  - The causal mask of each diagonal 128-block rides a DVE
    tensor_tensor_reduce (multiply by a 0/1 triangle; the reduce also
    produces that chunk's softmax row-sum), so the PE never sees a mask.
  - The softmax denominator is folded into V (the contraction index of
    P^T @ V is the softmax-row index).
  - Engine split: ACT does PSUM->fp16 copies + exp; DVE does the RoPE muls,
    diagonal masking and V row-scaling; Pool (slow software engine) gets
    only the RoPE add/subs.
"""

import sys

if "/opt/trn_rl_repo" not in sys.path:
    sys.path.insert(0, "/opt/trn_rl_repo")

import math

import numpy as np

import concourse.bacc as bacc
import concourse.tile as tile
from concourse import mybir

F32 = mybir.dt.float32
F16 = mybir.dt.float16
AF = mybir.ActivationFunctionType
ALU = mybir.AluOpType

B, S, D, H = 2, 2048, 512, 8
NCORES = 8
NT = S // 128  # 16 row-tiles per batch

_BUILT = None


def _interleave(a, b):
    """Merge unit lists evenly: spread b's units among a's."""
    if not a:
        return list(b)
    if not b:
        return list(a)
    out, fb, acc = [], len(b) / len(a), 0.0
    bi = 0
    for u in a:
        out.append(u)
        acc += fb
        while bi < len(b) and acc >= 1.0:
            out.append(b[bi])
            bi += 1
            acc -= 1.0
    out.extend(b[bi:])
    return out


def build_kernel(reps=1):
    nc = bacc.Bacc(trn_type="TRN2", target_bir_lowering=False, debug=False)

    qT_d = nc.dram_tensor("qT", [D, B * S], F16, kind="ExternalInput").ap()
    wq_d = nc.dram_tensor("wq", [D, D], F16, kind="ExternalInput").ap()
    wk_d = nc.dram_tensor("wk", [D, D], F16, kind="ExternalInput").ap()
    wv_d = nc.dram_tensor("wv", [D, D], F16, kind="ExternalInput").ap()
    wo_d = nc.dram_tensor("wo", [D, D], F16, kind="ExternalInput").ap()
    cos_d = nc.dram_tensor("cos2", [D // 2, S], F16, kind="ExternalInput").ap()
    sin_d = nc.dram_tensor("sin2", [D // 2, S], F16, kind="ExternalInput").ap()
    tri_d = nc.dram_tensor("tri01", [128, 512], F16, kind="ExternalInput").ap()
    outT_d = nc.dram_tensor("outT", [B, D, S], F32, kind="ExternalOutput").ap()

    with tile.TileContext(nc) as tc:
        with tc.tile_pool(name="const", bufs=1) as constp:
            wq_sb, wk_sb, wv_sb = [], [], []
            for nm, lst in (("wq", wq_sb), ("wk", wk_sb), ("wv", wv_sb)):
                for zt in range(4):
                    lst.append(constp.tile([128, D], F16, name=f"{nm}{zt}"))
            tri_sb = constp.tile([128, 512], F16, name="tri_sb")
            wo_sb = [constp.tile([128, D], F16, name=f"wo{zt}")
                     for zt in range(4)]
            cos_sb = [constp.tile([128, S], F16, name=f"cos{i}")
                      for i in range(2)]
            sin_sb = [constp.tile([128, S], F16, name=f"sin{i}")
                      for i in range(2)]
            # wq + trig go first; the rest are emitted mid-build so they don't
            # crowd the DMA queues ahead of the first projection's qT slices
            for zt in range(4):
                nc.sync.dma_start(out=wq_sb[zt],
                                  in_=wq_d[128 * zt : 128 * (zt + 1), :])
            for i in range(2):
                nc.sync.dma_start(out=cos_sb[i],
                                  in_=cos_d[128 * i : 128 * (i + 1), :])
                nc.sync.dma_start(out=sin_sb[i],
                                  in_=sin_d[128 * i : 128 * (i + 1), :])

            def deferred_loads(stage):
                if stage == 0:
                    for zt in range(4):
                        nc.sync.dma_start(
                            out=wk_sb[zt],
                            in_=wk_d[128 * zt : 128 * (zt + 1), :])
                elif stage == 1:
                    for zt in range(4):
                        nc.sync.dma_start(
                            out=wv_sb[zt],
                            in_=wv_d[128 * zt : 128 * (zt + 1), :])
                    nc.sync.dma_start(out=tri_sb, in_=tri_d)
                    for zt in range(4):
                        nc.sync.dma_start(
                            out=wo_sb[zt],
                            in_=wo_d[128 * zt : 128 * (zt + 1), :])

            for _rep in range(reps):
                for b in range(B):
                    _build_batch(
                        nc, tc, b, qT_d, wq_sb, wk_sb, wv_sb, cos_sb,
                        sin_sb, tri_sb, wo_sb, outT_d,
                        deferred_loads if (_rep == 0 and b == 0) else None,
                    )
    nc.compile()
    return nc


def _build_batch(nc, tc, b, qT_d, wq_sb, wk_sb, wv_sb, cos_sb, sin_sb,
                 tri_sb, wo_sb, outT_d, deferred_loads=None):
    with (
        tc.tile_pool(name=f"qk{b}", bufs=1) as qkpool,
        tc.tile_pool(name=f"v{b}", bufs=1) as vpool,
        tc.tile_pool(name=f"misc{b}", bufs=1) as mpool,
        tc.tile_pool(name=f"p{b}", bufs=1) as ppool,
    ):
        # rope'd Q^T, K^T: 4 partition-tiles each, [128, S] fp16
        QT = [qkpool.tile([128, S], F16, name=f"b{b}QT{i}", tag=f"QT{i}")
              for i in range(4)]
        KT = [qkpool.tile([128, S], F16, name=f"b{b}KT{i}", tag=f"KT{i}")
              for i in range(4)]
        V = [vpool.tile([128, D], F16, name=f"b{b}V{t}", tag=f"V{t}")
             for t in range(NT)]
        # per-(t, chunk) partial row sums, fp32
        rsp = mpool.tile([128, 4 * NT], F32, name=f"b{b}rsp")
        rsum = mpool.tile([128, NT], F32, name=f"b{b}rsum")
        rinv = mpool.tile([128, NT], F32, name=f"b{b}rinv")
        P = {}

        def proj_units(j, spool, tpool, psA):
            """Projections + rope for chunk j -> 8 emission units."""
            c0 = b * S + 512 * j
            sl = slice(512 * j, 512 * (j + 1))
            qs = []
            for zt in range(4):
                t_ = spool.tile([128, 512], F16, name=f"b{b}qs{zt}_{j}",
                                tag=f"qs{zt}")
                nc.sync.dma_start(
                    out=t_,
                    in_=qT_d[128 * zt : 128 * (zt + 1), c0 : c0 + 512])
                qs.append(t_)

            units = []
            for nm, wsb, dst in (("q", wq_sb, QT), ("k", wk_sb, KT)):
                for i in range(2):  # pair-half index
                    def u(nm=nm, wsb=wsb, dst=dst, i=i):
                        if deferred_loads is not None and nm == "k" \
                                and i == 0 and j == 0:
                            deferred_loads(0)
                        pe = psA.tile([128, 512], F32,
                                      name=f"b{b}{nm}pe{i}_{j}", tag="pe",
                                      space="PSUM")
                        po = psA.tile([128, 512], F32,
                                      name=f"b{b}{nm}po{i}_{j}", tag="po",
                                      space="PSUM")
                        for zt in range(4):
                            nc.tensor.matmul(
                                pe, wsb[zt][:, 128 * i : 128 * (i + 1)],
                                qs[zt], start=(zt == 0), stop=(zt == 3))
                        for zt in range(4):
                            nc.tensor.matmul(
                                po, wsb[zt][:, 128 * (i + 2) : 128 * (i + 3)],
                                qs[zt], start=(zt == 0), stop=(zt == 3))
                        pe16 = tpool.tile([128, 512], F16,
                                          name=f"pe16_{b}{nm}{i}{j}",
                                          tag="pe16")
                        po16 = tpool.tile([128, 512], F16,
                                          name=f"po16_{b}{nm}{i}{j}",
                                          tag="po16")
                        nc.scalar.copy(pe16, pe)
                        nc.scalar.copy(po16, po)
                        t1 = tpool.tile([128, 512], F16,
                                        name=f"t1_{b}{nm}{i}{j}", tag="t1")
                        t2 = tpool.tile([128, 512], F16,
                                        name=f"t2_{b}{nm}{i}{j}", tag="t2")
                        t3 = tpool.tile([128, 512], F16,
                                        name=f"t3_{b}{nm}{i}{j}", tag="t3")
                        t4 = tpool.tile([128, 512], F16,
                                        name=f"t4_{b}{nm}{i}{j}", tag="t4")
                        nc.vector.tensor_mul(t1, pe16, cos_sb[i][:, sl])
                        nc.vector.tensor_mul(t2, po16, sin_sb[i][:, sl])
                        nc.gpsimd.tensor_sub(dst[i][:, sl], t1, t2)
                        nc.vector.tensor_mul(t3, pe16, sin_sb[i][:, sl])
                        nc.vector.tensor_mul(t4, po16, cos_sb[i][:, sl])
                        nc.gpsimd.tensor_add(dst[i + 2][:, sl], t3, t4)
                    units.append(u)

            def after_qk():
                if deferred_loads is not None and j == 0:
                    deferred_loads(1)

            units.append(after_qk)
            # V projection (natural [s, d] layout; qT slices as stationary)
            for st in range(4):
                def uv(st=st):
                    pv = psA.tile([128, 512], F32, name=f"b{b}pv{j}_{st}",
                                  tag="pv", space="PSUM")
                    for zt in range(4):
                        nc.tensor.matmul(
                            pv, qs[zt][:, 128 * st : 128 * (st + 1)],
                            wv_sb[zt], start=(zt == 0), stop=(zt == 3))
                    nc.scalar.copy(V[4 * j + st], pv)
                units.append(uv)
            return units

        def score_units(j, psS):
            """Score row-tiles t = 4j..4j+3 -> one unit per 512-chunk."""
            units = []
            for t in range(4 * j, 4 * j + 4):
                Kt = 128 * (t + 1)
                nch = j + 1
                p_t = ppool.tile([128, Kt], F16, name=f"b{b}p{t}",
                                 tag=f"p{t}")
                P[t] = p_t
                for c in range(nch):
                    def uc(t=t, c=c, Kt=Kt, nch=nch, p_t=p_t):
                        w = min(512, Kt - 512 * c)
                        ps = psS.tile([128, 512], F32,
                                      name=f"b{b}ps{t}_{c}", tag="s",
                                      space="PSUM")
                        last = c == nch - 1
                        for dt_ in range(4):
                            nc.tensor.matmul(
                                ps[:, :w],
                                QT[dt_][:, 128 * t : 128 * (t + 1)],
                                KT[dt_][:, 512 * c : 512 * c + w],
                                start=(dt_ == 0), stop=(dt_ == 3))
                        psl = p_t[:, 512 * c : 512 * c + w]
                        slot = rsp[:, 4 * t + c : 4 * t + c + 1]
                        if not last:
                            nc.scalar.activation(psl, ps[:, :w], AF.Exp,
                                                 accum_out=slot)
                        else:
                            # diagonal block: exp, then 0/1-triangle mask and
                            # row-sum on the DVE (no PE mask matmul)
                            nc.scalar.activation(psl, ps[:, :w], AF.Exp)
                            nc.vector.tensor_mul(psl, psl,
                                                 tri_sb[:, 512 - w : 512])
                            nc.vector.tensor_reduce(
                                slot, psl, mybir.AxisListType.X, ALU.add)
                            if nch == 1:
                                nc.vector.reciprocal(
                                    rinv[:, t : t + 1], slot)
                            else:
                                nc.vector.tensor_reduce(
                                    rsum[:, t : t + 1],
                                    rsp[:, 4 * t : 4 * t + nch],
                                    mybir.AxisListType.X, ALU.add)
                                nc.vector.reciprocal(rinv[:, t : t + 1],
                                                     rsum[:, t : t + 1])
                            # fold softmax denominator into V rows
                            nc.vector.tensor_scalar_mul(
                                V[t], V[t], rinv[:, t : t + 1])
                    units.append(uc)
            return units

        def pv_group(j, order, psPV, opool):
            """P^T V accumulation for output chunk j -> unit per t."""
            po = [psPV.tile([128, 512], F32, name=f"b{b}po{j}_{dt_}",
                            tag=f"o{dt_}", space="PSUM")
                  for dt_ in range(4)]
            units = []
            for t in order:
                def ut(t=t):
                    n = min(512, 128 * (t + 1) - 512 * j)
                    for dt_ in range(4):
                        nc.tensor.matmul(
                            po[dt_][:, :n],
                            V[t][:, 128 * dt_ : 128 * (dt_ + 1)],
                            P[t][:, 512 * j : 512 * j + n],
                            start=(t == order[0]), stop=(t == order[-1]))
                units.append(ut)

            def drain():
                oT = []
                for dt_ in range(4):
                    o_ = opool.tile([128, 512], F16,
                                    name=f"b{b}oT{j}_{dt_}", tag=f"oT{dt_}")
                    nc.scalar.copy(o_, po[dt_])
                    oT.append(o_)
                return oT
            return units, drain

        def wo_units(j, oT, psWo, opool):
            units = []
            for dot in range(4):
                def ud(dot=dot):
                    pf = psWo.tile([128, 512], F32, name=f"b{b}pf{j}_{dot}",
                                   tag="f", space="PSUM")
                    for dit in range(4):
                        nc.tensor.matmul(
                            pf, wo_sb[dit][:, 128 * dot : 128 * (dot + 1)],
                            oT[dit], start=(dit == 0), stop=(dit == 3))
                    of = opool.tile([128, 512], F32, name=f"b{b}of{j}_{dot}",
                                    tag="of")
                    nc.scalar.copy(of, pf)
                    nc.sync.dma_start(
                        out=outT_d[b, 128 * dot : 128 * (dot + 1),
                                   512 * j : 512 * (j + 1)],
                        in_=of)
                units.append(ud)
            return units

        with tc.tile_pool(name=f"psS{b}", bufs=2, space="PSUM") as psS:
            # ---- pipelined: projections(j) interleaved with scores(j-1) ----
            with (
                tc.tile_pool(name=f"st{b}", bufs=2) as spool,
                tc.tile_pool(name=f"t{b}", bufs=2) as tpool,
                tc.tile_pool(name=f"psA{b}", bufs=2, space="PSUM") as psA,
            ):
                for j in range(4):
                    pu = proj_units(j, spool, tpool, psA)
                    su = score_units(j - 1, psS) if j > 0 else []
                    for u in _interleave(pu, su):
                        u()
            # ---- tail: scores(3) interleaved with P^T V of chunk 0 --------
            with (
                tc.tile_pool(name=f"o{b}", bufs=2) as opool,
                tc.tile_pool(name=f"psPV{b}", bufs=1, space="PSUM") as psPV,
                tc.tile_pool(name=f"psWo{b}", bufs=2, space="PSUM") as psWo,
            ):
                su = score_units(3, psS)
                # chunk 0: first matmul must cover the whole bank, so start
                # with the earliest full-width tile (t=3); finish with
                # t=12..15 which become ready as the score wave completes
                order0 = [3] + list(range(4, 12)) + [2, 1, 0] + \
                    list(range(12, NT))
                pv0, drain0 = pv_group(0, order0, psPV, opool)
                # interleave: scores lead (they unblock everything)
                for u in _interleave(su, pv0[:12]):
                    u()
                for u in pv0[12:]:
                    u()
                oT0 = drain0()
                wo0 = wo_units(0, oT0, psWo, opool)
                for j in range(1, 4):
                    order = [4 * j + 3] + list(range(4 * j + 4, NT)) + [
                        4 * j + 2, 4 * j + 1, 4 * j]
                    pvj, drainj = pv_group(j, order, psPV, opool)
                    prev_wo = wo0 if j == 1 else wo_units(j - 1, oTprev,
                                                          psWo, opool)
                    for u in _interleave(pvj, prev_wo):
                        u()
                    oTprev = drainj()
                for u in wo_units(3, oTprev, psWo, opool):
                    u()


def _host_inputs(q, W_q, W_k, W_v, W_o):
    """Build the 8 per-core input maps."""
    scale = 1.0 / math.sqrt(D)
    perm = np.concatenate([np.arange(0, D, 2), np.arange(1, D, 2)])

    qT = np.ascontiguousarray(q.reshape(B * S, D).T).astype(np.float16)

    # trig tables, float32 pipeline mirroring the reference's jnp math
    inv_freq = (1.0 / (10000.0 ** (np.arange(0, D, 2, dtype=np.float32) /
                                   np.float32(D)))).astype(np.float32)
    ang = (np.arange(S, dtype=np.float32)[:, None] * inv_freq[None, :])
    cos2 = np.ascontiguousarray(np.cos(ang, dtype=np.float32).T).astype(
        np.float16)
    sin2 = np.ascontiguousarray(np.sin(ang, dtype=np.float32).T).astype(
        np.float16)

    # [ones(384) | lower-triangle] 0/1 mask; slicing [:, 512-w:] fits any
    # chunk width w with the diagonal 128-block in the last 128 columns
    r = np.arange(128)[:, None]
    c = np.arange(128)[None, :]
    tri01 = np.concatenate(
        [np.ones((128, 384), np.float16),
         (c <= r).astype(np.float16)], axis=1)

    in_maps = []
    for h in range(NCORES):
        in_maps.append({
            "qT": qT,
            "wq": np.ascontiguousarray((W_q[h] * scale)[:, perm]).astype(
                np.float16),
            "wk": np.ascontiguousarray(W_k[h][:, perm]).astype(np.float16),
            "wv": np.ascontiguousarray(W_v[h]).astype(np.float16),
            "wo": np.ascontiguousarray(W_o[D * h : D * (h + 1), :]).astype(
                np.float16),
            "cos2": cos2,
            "sin2": sin2,
            "tri01": tri01,
        })
    return in_maps


def kernel(q, W_q, W_k, W_v, W_o):
    from concourse.bass_utils import run_bass_kernel_spmd

    global _BUILT
    q = np.asarray(q, dtype=np.float32)
    W_q = np.asarray(W_q, dtype=np.float32)
    W_k = np.asarray(W_k, dtype=np.float32)
    W_v = np.asarray(W_v, dtype=np.float32)
    W_o = np.asarray(W_o, dtype=np.float32)

    if _BUILT is None:
        _BUILT = build_kernel()
    nc = _BUILT

    in_maps = _host_inputs(q, W_q, W_k, W_v, W_o)
    res = run_bass_kernel_spmd(nc, in_maps, list(range(NCORES)))

    acc = np.zeros((B, S, D), dtype=np.float64)
    for h in range(NCORES):
        acc += res.results[h]["outT"].transpose(0, 2, 1)
    return acc.astype(np.float32)


# revision 6
# speedup vs baseline: 2.9348x; 2.9348x over previous
"""Trainium2 Bass kernel for nn_Attention_73031623901249.

Multi-head attention with per-head 512x512 projections, interleaved RoPE,
causal softmax, a transposed P^T @ V contraction, and an output projection.

Sharding: one head per NeuronCore (H == 8 == n_cores). Each core computes its
head's full attention plus its slice of the W_o projection; the host sums the
8 partial outputs.

Layout/precision choices:
  - The V projection is fused into the output projection: the reference
    computes P^T (q W_v) W_o, which equals (q^T P)^T (W_v W_o). The host
    precomputes W_vo = W_v @ W_o per head; the kernel contracts M = q^T P
    directly (same triangular loop the P^T V product would need) and then
    applies W_vo. This removes the entire V projection from the PE.
  - Everything on SBUF is fp16 (inputs are cast host-side): matmul moving
    operands run at 1 col/cycle at any width, and DVE elementwise ops hit
    the 2x fast path (all-SBUF, 2-byte, packed). PSUM stays fp32.
  - q is fed twice: transposed qT [D, B*S] (moving operand of the Q/K
    projections) and natural qn [B*S, D] (stationary tiles of q^T P).
    W_q / W_k columns are permuted even/odd -> [evens | odds] (W_q
    pre-scaled by 1/sqrt(D)) so interleaved RoPE becomes elementwise ops on
    partition-aligned halves.
  - The kernel is software-pipelined: score row-tiles for chunk j-1 are
    interleaved (at emission granularity) with projections+RoPE of chunk j,
    and the final score wave overlaps the start of the q^T P phase. Scores
    stream through 512-wide PSUM chunks at exact causal width, exp'd (ACT)
    straight to fp16 P tiles.
  - The causal mask of each diagonal 128-block rides DVE ops (multiply by a
    0/1 triangle + row-sum), so the PE never sees a mask.
  - The softmax denominator is folded into the qn rows (the contraction
    index of q^T P is the softmax-row index).
  - Engine split: ACT does PSUM->fp16 copies + exp; DVE does the RoPE muls,
    diagonal masking and qn row-scaling; Pool (slow software engine) gets
    only the RoPE add/subs.
"""

import sys

if "/opt/trn_rl_repo" not in sys.path:
    sys.path.insert(0, "/opt/trn_rl_repo")

import math

import numpy as np

import concourse.bacc as bacc
import concourse.tile as tile
from concourse import mybir

F32 = mybir.dt.float32
F16 = mybir.dt.float16
AF = mybir.ActivationFunctionType
ALU = mybir.AluOpType

B, S, D, H = 2, 2048, 512, 8
NCORES = 8
NT = S // 128  # 16 row-tiles per batch

_BUILT = None


def _interleave(a, b):
    """Merge unit lists evenly: spread b's units among a's."""
    if not a:
        return list(b)
    if not b:
        return list(a)
    out, fb, acc = [], len(b) / len(a), 0.0
    bi = 0
    for u in a:
        out.append(u)
        acc += fb
        while bi < len(b) and acc >= 1.0:
            out.append(b[bi])
            bi += 1
            acc -= 1.0
    out.extend(b[bi:])
    return out


def build_kernel(reps=1):
    nc = bacc.Bacc(trn_type="TRN2", target_bir_lowering=False, debug=False)

    qT_d = nc.dram_tensor("qT", [D, B * S], F16, kind="ExternalInput").ap()
    qn_d = nc.dram_tensor("qn", [B * S, D], F16, kind="ExternalInput").ap()
    wq_d = nc.dram_tensor("wq", [D, D], F16, kind="ExternalInput").ap()
    wk_d = nc.dram_tensor("wk", [D, D], F16, kind="ExternalInput").ap()
    wvo_d = nc.dram_tensor("wvo", [D, D], F16, kind="ExternalInput").ap()
    cos_d = nc.dram_tensor("cos2", [D // 2, S], F16, kind="ExternalInput").ap()
    sin_d = nc.dram_tensor("sin2", [D // 2, S], F16, kind="ExternalInput").ap()
    tri_d = nc.dram_tensor("tri01", [128, 512], F16, kind="ExternalInput").ap()
    outT_d = nc.dram_tensor("outT", [B, D, S], F32, kind="ExternalOutput").ap()

    with tile.TileContext(nc) as tc:
        with tc.tile_pool(name="const", bufs=1) as constp:
            wq_sb, wk_sb = [], []
            for nm, lst in (("wq", wq_sb), ("wk", wk_sb)):
                for zt in range(4):
                    lst.append(constp.tile([128, D], F16, name=f"{nm}{zt}"))
            tri_sb = constp.tile([128, 512], F16, name="tri_sb")
            wvo_sb = [constp.tile([128, D], F16, name=f"wvo{zt}")
                      for zt in range(4)]
            cos_sb = [constp.tile([128, S], F16, name=f"cos{i}")
                      for i in range(2)]
            sin_sb = [constp.tile([128, S], F16, name=f"sin{i}")
                      for i in range(2)]
            # wq + trig go first; the rest are emitted mid-build so they don't
            # crowd the DMA queues ahead of the first projection's qT slices
            for zt in range(4):
                nc.sync.dma_start(out=wq_sb[zt],
                                  in_=wq_d[128 * zt : 128 * (zt + 1), :])
            for i in range(2):
                nc.sync.dma_start(out=cos_sb[i],
                                  in_=cos_d[128 * i : 128 * (i + 1), :])
                nc.sync.dma_start(out=sin_sb[i],
                                  in_=sin_d[128 * i : 128 * (i + 1), :])

            def deferred_loads(stage):
                if stage == 0:
                    for zt in range(4):
                        nc.sync.dma_start(
                            out=wk_sb[zt],
                            in_=wk_d[128 * zt : 128 * (zt + 1), :])
                elif stage == 1:
                    nc.sync.dma_start(out=tri_sb, in_=tri_d)
                    for zt in range(4):
                        nc.sync.dma_start(
                            out=wvo_sb[zt],
                            in_=wvo_d[128 * zt : 128 * (zt + 1), :])

            for _rep in range(reps):
                for b in range(B):
                    _build_batch(
                        nc, tc, b, qT_d, qn_d, wq_sb, wk_sb, cos_sb,
                        sin_sb, tri_sb, wvo_sb, outT_d,
                        deferred_loads if (_rep == 0 and b == 0) else None,
                    )
    nc.compile()
    return nc


def _build_batch(nc, tc, b, qT_d, qn_d, wq_sb, wk_sb, cos_sb, sin_sb,
                 tri_sb, wvo_sb, outT_d, deferred_loads=None):
    with (
        tc.tile_pool(name=f"qk{b}", bufs=1) as qkpool,
        tc.tile_pool(name=f"qn{b}", bufs=1) as qnpool,
        tc.tile_pool(name=f"misc{b}", bufs=1) as mpool,
        tc.tile_pool(name=f"p{b}", bufs=1) as ppool,
    ):
        # rope'd Q^T, K^T: 4 partition-tiles each, [128, S] fp16
        QT = [qkpool.tile([128, S], F16, name=f"b{b}QT{i}", tag=f"QT{i}")
              for i in range(4)]
        KT = [qkpool.tile([128, S], F16, name=f"b{b}KT{i}", tag=f"KT{i}")
              for i in range(4)]
        # natural-layout q row-tiles (stationary side of q^T P)
        QN = [qnpool.tile([128, D], F16, name=f"b{b}qn{t}", tag=f"qn{t}")
              for t in range(NT)]
        # per-(t, chunk) partial row sums, fp32
        rsp = mpool.tile([128, 4 * NT], F32, name=f"b{b}rsp")
        rsum = mpool.tile([128, NT], F32, name=f"b{b}rsum")
        rinv = mpool.tile([128, NT], F32, name=f"b{b}rinv")
        P = {}

        def proj_units(j, spool, tpool, psA):
            """Projections + rope for chunk j -> emission units."""
            c0 = b * S + 512 * j
            sl = slice(512 * j, 512 * (j + 1))
            qs = []
            for zt in range(4):
                t_ = spool.tile([128, 512], F16, name=f"b{b}qs{zt}_{j}",
                                tag=f"qs{zt}")
                nc.sync.dma_start(
                    out=t_,
                    in_=qT_d[128 * zt : 128 * (zt + 1), c0 : c0 + 512])
                qs.append(t_)
            for st in range(4):
                t = 4 * j + st
                nc.sync.dma_start(
                    out=QN[t],
                    in_=qn_d[c0 + 128 * st : c0 + 128 * (st + 1), :])

            units = []
            for nm, wsb, dst in (("q", wq_sb, QT), ("k", wk_sb, KT)):
                for i in range(2):  # pair-half index
                    def u(nm=nm, wsb=wsb, dst=dst, i=i):
                        if deferred_loads is not None and nm == "k" \
                                and i == 0 and j == 0:
                            deferred_loads(0)
                        pe = psA.tile([128, 512], F32,
                                      name=f"b{b}{nm}pe{i}_{j}", tag="pe",
                                      space="PSUM")
                        po = psA.tile([128, 512], F32,
                                      name=f"b{b}{nm}po{i}_{j}", tag="po",
                                      space="PSUM")
                        for zt in range(4):
                            nc.tensor.matmul(
                                pe, wsb[zt][:, 128 * i : 128 * (i + 1)],
                                qs[zt], start=(zt == 0), stop=(zt == 3))
                        for zt in range(4):
                            nc.tensor.matmul(
                                po, wsb[zt][:, 128 * (i + 2) : 128 * (i + 3)],
                                qs[zt], start=(zt == 0), stop=(zt == 3))
                        pe16 = tpool.tile([128, 512], F16,
                                          name=f"pe16_{b}{nm}{i}{j}",
                                          tag="pe16")
                        po16 = tpool.tile([128, 512], F16,
                                          name=f"po16_{b}{nm}{i}{j}",
                                          tag="po16")
                        nc.scalar.copy(pe16, pe)
                        nc.scalar.copy(po16, po)
                        t1 = tpool.tile([128, 512], F16,
                                        name=f"t1_{b}{nm}{i}{j}", tag="t1")
                        t2 = tpool.tile([128, 512], F16,
                                        name=f"t2_{b}{nm}{i}{j}", tag="t2")
                        t3 = tpool.tile([128, 512], F16,
                                        name=f"t3_{b}{nm}{i}{j}", tag="t3")
                        t4 = tpool.tile([128, 512], F16,
                                        name=f"t4_{b}{nm}{i}{j}", tag="t4")
                        nc.vector.tensor_mul(t1, pe16, cos_sb[i][:, sl])
                        nc.vector.tensor_mul(t2, po16, sin_sb[i][:, sl])
                        nc.gpsimd.tensor_sub(dst[i][:, sl], t1, t2)
                        nc.vector.tensor_mul(t3, pe16, sin_sb[i][:, sl])
                        nc.vector.tensor_mul(t4, po16, cos_sb[i][:, sl])
                        nc.gpsimd.tensor_add(dst[i + 2][:, sl], t3, t4)
                    units.append(u)

            def after_qk():
                if deferred_loads is not None and j == 0:
                    deferred_loads(1)

            units.append(after_qk)
            return units

        def score_units(j, psS):
            """Score row-tiles t = 4j..4j+3 -> one unit per 512-chunk."""
            units = []
            for t in range(4 * j, 4 * j + 4):
                Kt = 128 * (t + 1)
                nch = j + 1
                p_t = ppool.tile([128, Kt], F16, name=f"b{b}p{t}",
                                 tag=f"p{t}")
                P[t] = p_t
                for c in range(nch):
                    def uc(t=t, c=c, Kt=Kt, nch=nch, p_t=p_t):
                        w = min(512, Kt - 512 * c)
                        ps = psS.tile([128, 512], F32,
                                      name=f"b{b}ps{t}_{c}", tag="s",
                                      space="PSUM")
                        last = c == nch - 1
                        for dt_ in range(4):
                            nc.tensor.matmul(
                                ps[:, :w],
                                QT[dt_][:, 128 * t : 128 * (t + 1)],
                                KT[dt_][:, 512 * c : 512 * c + w],
                                start=(dt_ == 0), stop=(dt_ == 3))
                        psl = p_t[:, 512 * c : 512 * c + w]
                        slot = rsp[:, 4 * t + c : 4 * t + c + 1]
                        if not last:
                            nc.scalar.activation(psl, ps[:, :w], AF.Exp,
                                                 accum_out=slot)
                        else:
                            # diagonal block: exp, then 0/1-triangle mask and
                            # row-sum on the DVE (no PE mask matmul)
                            nc.scalar.activation(psl, ps[:, :w], AF.Exp)
                            nc.vector.tensor_mul(psl, psl,
                                                 tri_sb[:, 512 - w : 512])
                            nc.vector.tensor_reduce(
                                slot, psl, mybir.AxisListType.X, ALU.add)
                            if nch == 1:
                                nc.vector.reciprocal(
                                    rinv[:, t : t + 1], slot)
                            else:
                                nc.vector.tensor_reduce(
                                    rsum[:, t : t + 1],
                                    rsp[:, 4 * t : 4 * t + nch],
                                    mybir.AxisListType.X, ALU.add)
                                nc.vector.reciprocal(rinv[:, t : t + 1],
                                                     rsum[:, t : t + 1])
                            # fold softmax denominator into the qn rows
                            # (contraction index of q^T P)
                            nc.vector.tensor_scalar_mul(
                                QN[t], QN[t], rinv[:, t : t + 1])
                    units.append(uc)
            return units

        def qp_group(j, order, psPV, opool):
            """M = q^T P accumulation for output chunk j -> unit per t."""
            po = [psPV.tile([128, 512], F32, name=f"b{b}po{j}_{dt_}",
                            tag=f"o{dt_}", space="PSUM")
                  for dt_ in range(4)]
            units = []
            for t in order:
                def ut(t=t):
                    n = min(512, 128 * (t + 1) - 512 * j)
                    for dt_ in range(4):
                        nc.tensor.matmul(
                            po[dt_][:, :n],
                            QN[t][:, 128 * dt_ : 128 * (dt_ + 1)],
                            P[t][:, 512 * j : 512 * j + n],
                            start=(t == order[0]), stop=(t == order[-1]))
                units.append(ut)

            def drain():
                oT = []
                for dt_ in range(4):
                    o_ = opool.tile([128, 512], F16,
                                    name=f"b{b}oT{j}_{dt_}", tag=f"oT{dt_}")
                    nc.scalar.copy(o_, po[dt_])
                    oT.append(o_)
                return oT
            return units, drain

        def wo_units(j, oT, psWo, opool):
            units = []
            for dot in range(4):
                def ud(dot=dot):
                    pf = psWo.tile([128, 512], F32, name=f"b{b}pf{j}_{dot}",
                                   tag="f", space="PSUM")
                    for dit in range(4):
                        nc.tensor.matmul(
                            pf, wvo_sb[dit][:, 128 * dot : 128 * (dot + 1)],
                            oT[dit], start=(dit == 0), stop=(dit == 3))
                    of = opool.tile([128, 512], F32, name=f"b{b}of{j}_{dot}",
                                    tag="of")
                    nc.scalar.copy(of, pf)
                    nc.sync.dma_start(
                        out=outT_d[b, 128 * dot : 128 * (dot + 1),
                                   512 * j : 512 * (j + 1)],
                        in_=of)
                units.append(ud)
            return units

        with tc.tile_pool(name=f"psS{b}", bufs=2, space="PSUM") as psS:
            # ---- pipelined: projections(j) interleaved with scores(j-1) ----
            with (
                tc.tile_pool(name=f"st{b}", bufs=2) as spool,
                tc.tile_pool(name=f"t{b}", bufs=2) as tpool,
                tc.tile_pool(name=f"psA{b}", bufs=3, space="PSUM") as psA,
            ):
                for j in range(4):
                    pu = proj_units(j, spool, tpool, psA)
                    su = score_units(j - 1, psS) if j > 0 else []
                    for u in _interleave(pu, su):
                        u()
            # ---- tail: scores(3) interleaved with q^T P of chunk 0 --------
            with (
                tc.tile_pool(name=f"o{b}", bufs=2) as opool,
                tc.tile_pool(name=f"psPV{b}", bufs=1, space="PSUM") as psPV,
                tc.tile_pool(name=f"psWo{b}", bufs=2, space="PSUM") as psWo,
            ):
                su = score_units(3, psS)
                # chunk 0: first matmul must cover the whole bank, so start
                # with the earliest full-width tile (t=3); finish with
                # t=12..15 which become ready as the score wave completes
                order0 = [3] + list(range(4, 12)) + [2, 1, 0] + \
                    list(range(12, NT))
                pv0, drain0 = qp_group(0, order0, psPV, opool)
                # interleave: scores lead (they unblock everything)
                for u in _interleave(su, pv0[:12]):
                    u()
                for u in pv0[12:]:
                    u()
                oT0 = drain0()
                wo0 = wo_units(0, oT0, psWo, opool)
                for j in range(1, 4):
                    order = [4 * j + 3] + list(range(4 * j + 4, NT)) + [
                        4 * j + 2, 4 * j + 1, 4 * j]
                    pvj, drainj = qp_group(j, order, psPV, opool)
                    prev_wo = wo0 if j == 1 else wo_units(j - 1, oTprev,
                                                          psWo, opool)
                    for u in _interleave(pvj, prev_wo):
                        u()
                    oTprev = drainj()
                for u in wo_units(3, oTprev, psWo, opool):
                    u()


def _host_inputs(q, W_q, W_k, W_v, W_o):
    """Build the 8 per-core input maps."""
    scale = 1.0 / math.sqrt(D)
    perm = np.concatenate([np.arange(0, D, 2), np.arange(1, D, 2)])

    q2 = q.reshape(B * S, D)
    qT = np.ascontiguousarray(q2.T).astype(np.float16)
    qn = np.ascontiguousarray(q2).astype(np.float16)

    # trig tables, float32 pipeline mirroring the reference's jnp math
    inv_freq = (1.0 / (10000.0 ** (np.arange(0, D, 2, dtype=np.float32) /
                                   np.float32(D)))).astype(np.float32)
    ang = (np.arange(S, dtype=np.float32)[:, None] * inv_freq[None, :])
    cos2 = np.ascontiguousarray(np.cos(ang, dtype=np.float32).T).astype(
        np.float16)
    sin2 = np.ascontiguousarray(np.sin(ang, dtype=np.float32).T).astype(
        np.float16)

    # [ones(384) | lower-triangle] 0/1 mask; slicing [:, 512-w:] fits any
    # chunk width w with the diagonal 128-block in the last 128 columns
    r = np.arange(128)[:, None]
    c = np.arange(128)[None, :]
    tri01 = np.concatenate(
        [np.ones((128, 384), np.float16),
         (c <= r).astype(np.float16)], axis=1)

    in_maps = []
    for h in range(NCORES):
        wvo = W_v[h].astype(np.float32) @ W_o[D * h : D * (h + 1), :].astype(
            np.float32)
        in_maps.append({
            "qT": qT,
            "qn": qn,
            "wq": np.ascontiguousarray((W_q[h] * scale)[:, perm]).astype(
                np.float16),
            "wk": np.ascontiguousarray(W_k[h][:, perm]).astype(np.float16),
            "wvo": np.ascontiguousarray(wvo).astype(np.float16),
            "cos2": cos2,
            "sin2": sin2,
            "tri01": tri01,
        })
    return in_maps


def kernel(q, W_q, W_k, W_v, W_o):
    from concourse.bass_utils import run_bass_kernel_spmd

    global _BUILT
    q = np.asarray(q, dtype=np.float32)
    W_q = np.asarray(W_q, dtype=np.float32)
    W_k = np.asarray(W_k, dtype=np.float32)
    W_v = np.asarray(W_v, dtype=np.float32)
    W_o = np.asarray(W_o, dtype=np.float32)

    if _BUILT is None:
        _BUILT = build_kernel()
    nc = _BUILT

    in_maps = _host_inputs(q, W_q, W_k, W_v, W_o)
    res = run_bass_kernel_spmd(nc, in_maps, list(range(NCORES)))

    acc = np.zeros((B, S, D), dtype=np.float64)
    for h in range(NCORES):
        acc += res.results[h]["outT"].transpose(0, 2, 1)
    return acc.astype(np.float32)


# revision 8
# speedup vs baseline: 3.5145x; 1.1975x over previous
"""Trainium2 Bass kernel for nn_Attention_73031623901249.

Multi-head attention with per-head 512x512 projections, interleaved RoPE,
causal softmax, a transposed P^T @ V contraction, and an output projection.

Sharding: one head per NeuronCore (H == 8 == n_cores). Each core computes its
head's full attention plus its slice of the W_o projection; the host sums the
8 partial outputs.

Layout/precision choices:
  - The V projection is fused into the output projection: the reference
    computes P^T (q W_v) W_o, which equals (q^T P)^T (W_v W_o). The host
    precomputes W_vo = W_v @ W_o per head; the kernel contracts M = q^T P
    directly (same triangular loop the P^T V product would need) and then
    applies W_vo. This removes the entire V projection from the PE.
  - Everything on SBUF is fp16 (inputs are cast host-side): matmul moving
    operands run at 1 col/cycle at any width, and DVE elementwise ops hit
    the 2x fast path (all-SBUF, 2-byte, packed). PSUM stays fp32.
  - q is fed twice: transposed qT [D, B*S] (moving operand of the Q/K
    projections) and natural qn [B*S, D] (stationary tiles of q^T P).
    W_q / W_k columns are permuted even/odd -> [evens | odds] (W_q
    pre-scaled by 1/sqrt(D)) so interleaved RoPE becomes elementwise ops on
    partition-aligned halves.
  - The kernel is software-pipelined: score row-tiles for chunk j-1 are
    interleaved (at emission granularity) with projections+RoPE of chunk j,
    the final score wave overlaps the start of q^T P, and the M = q^T P
    accumulation runs as two 2-bank passes (dt 0/1, then dt 2/3) so
    successive output chunks overlap and the PSUM->SBUF drains hide under
    W_vo matmuls. SBUF pools are global: the next batch's first q tiles are
    prefetched during the previous batch's tail, and QT/KT are
    double-buffered across batches.
  - Scores stream through 512-wide PSUM chunks at exact causal width,
    exp'd (ACT) straight to fp16 P tiles. The causal mask of each diagonal
    128-block rides DVE ops (multiply by a 0/1 triangle + row-sum), so the
    PE never sees a mask. The softmax denominator is folded into the qn
    rows (the contraction index of q^T P is the softmax-row index).
  - Engine split: ACT does PSUM->fp16 copies + exp; DVE does the RoPE muls,
    diagonal masking and qn row-scaling; Pool (slow software engine) gets
    only the RoPE add/subs.
"""

import sys

if "/opt/trn_rl_repo" not in sys.path:
    sys.path.insert(0, "/opt/trn_rl_repo")

import math

import numpy as np

import concourse.bacc as bacc
import concourse.tile as tile
from concourse import mybir

F32 = mybir.dt.float32
F16 = mybir.dt.float16
AF = mybir.ActivationFunctionType
ALU = mybir.AluOpType

B, S, D, H = 2, 2048, 512, 8
NCORES = 8
NT = S // 128  # 16 row-tiles per batch

_BUILT = None


def _interleave(a, b):
    """Merge unit lists evenly: spread b's units among a's."""
    if not a:
        return list(b)
    if not b:
        return list(a)
    out, fb, acc = [], len(b) / len(a), 0.0
    bi = 0
    for u in a:
        out.append(u)
        acc += fb
        while bi < len(b) and acc >= 1.0:
            out.append(b[bi])
            bi += 1
            acc -= 1.0
    out.extend(b[bi:])
    return out


def build_kernel(reps=1):
    nc = bacc.Bacc(trn_type="TRN2", target_bir_lowering=False, debug=False)

    qT_d = nc.dram_tensor("qT", [D, B * S], F16, kind="ExternalInput").ap()
    qn_d = nc.dram_tensor("qn", [B * S, D], F16, kind="ExternalInput").ap()
    wq_d = nc.dram_tensor("wq", [D, D], F16, kind="ExternalInput").ap()
    wk_d = nc.dram_tensor("wk", [D, D], F16, kind="ExternalInput").ap()
    wvo_d = nc.dram_tensor("wvo", [D, D], F16, kind="ExternalInput").ap()
    cos_d = nc.dram_tensor("cos2", [D // 2, S], F16, kind="ExternalInput").ap()
    sin_d = nc.dram_tensor("sin2", [D // 2, S], F16, kind="ExternalInput").ap()
    tri_d = nc.dram_tensor("tri01", [128, 512], F16, kind="ExternalInput").ap()
    outT_d = nc.dram_tensor("outT", [B, D, S], F32, kind="ExternalOutput").ap()

    with tile.TileContext(nc) as tc:
        with (
            tc.tile_pool(name="const", bufs=1) as constp,
            tc.tile_pool(name="qk", bufs=2) as qkpool,
            tc.tile_pool(name="qn", bufs=1) as qnpool,
            tc.tile_pool(name="misc", bufs=1) as mpool,
            tc.tile_pool(name="p", bufs=1) as ppool,
            tc.tile_pool(name="st", bufs=2) as spool,
            tc.tile_pool(name="t", bufs=2) as tpool,
            tc.tile_pool(name="o", bufs=2) as opool,
        ):
            pools = dict(qk=qkpool, qn=qnpool, misc=mpool, p=ppool,
                         st=spool, t=tpool, o=opool)
            wq_sb, wk_sb = [], []
            for nm, lst in (("wq", wq_sb), ("wk", wk_sb)):
                for zt in range(4):
                    lst.append(constp.tile([128, D], F16, name=f"{nm}{zt}"))
            tri_sb = constp.tile([128, 512], F16, name="tri_sb")
            wvo_sb = [constp.tile([128, D], F16, name=f"wvo{zt}")
                      for zt in range(4)]
            cos_sb = [constp.tile([128, S], F16, name=f"cos{i}")
                      for i in range(2)]
            sin_sb = [constp.tile([128, S], F16, name=f"sin{i}")
                      for i in range(2)]
            consts = dict(wq=wq_sb, wk=wk_sb, wvo=wvo_sb, tri=tri_sb,
                          cos=cos_sb, sin=sin_sb)

            def fetch_q(b, j):
                """Create + DMA chunk (b, j)'s qT slices and qn row-tiles."""
                c0 = b * S + 512 * j
                qs = []
                for zt in range(4):
                    t_ = spool.tile([128, 512], F16, name=f"b{b}qs{zt}_{j}",
                                    tag=f"qs{zt}")
                    nc.sync.dma_start(
                        out=t_,
                        in_=qT_d[128 * zt : 128 * (zt + 1), c0 : c0 + 512])
                    qs.append(t_)
                qn = []
                for st in range(4):
                    t_ = qnpool.tile([128, D], F16,
                                     name=f"b{b}qn{4 * j + st}",
                                     tag=f"qn{4 * j + st}")
                    nc.sync.dma_start(
                        out=t_,
                        in_=qn_d[c0 + 128 * st : c0 + 128 * (st + 1), :])
                    qn.append(t_)
                return qs, qn

            # startup order: first chunk's q before anything bulky, then
            # wq + trig; wk and the tail-phase consts ride deferred_loads
            q0 = fetch_q(0, 0)
            for zt in range(4):
                nc.sync.dma_start(out=wq_sb[zt],
                                  in_=wq_d[128 * zt : 128 * (zt + 1), :])
            for i in range(2):
                nc.sync.dma_start(out=cos_sb[i],
                                  in_=cos_d[128 * i : 128 * (i + 1), :])
                nc.sync.dma_start(out=sin_sb[i],
                                  in_=sin_d[128 * i : 128 * (i + 1), :])

            def deferred_loads(stage):
                if stage == 0:
                    for zt in range(4):
                        nc.sync.dma_start(
                            out=wk_sb[zt],
                            in_=wk_d[128 * zt : 128 * (zt + 1), :])
                elif stage == 1:
                    nc.sync.dma_start(out=tri_sb, in_=tri_d)
                    for zt in range(4):
                        nc.sync.dma_start(
                            out=wvo_sb[zt],
                            in_=wvo_d[128 * zt : 128 * (zt + 1), :])

            for _rep in range(reps):
                for b in range(B):
                    first = _rep == 0 and b == 0
                    nxt = (0 if b == B - 1 else b + 1)
                    q0 = _build_batch(
                        nc, tc, b, pools, consts, fetch_q, q0, outT_d,
                        deferred_loads if first else None,
                        # prefetch the next batch's first chunk mid-tail
                        # (skip on the very last emitted batch)
                        None if (_rep == reps - 1 and b == B - 1) else nxt,
                    )
    nc.compile()
    return nc


def _build_batch(nc, tc, b, pools, consts, fetch_q, q0, outT_d,
                 deferred_loads=None, prefetch_b=None):
    qkpool, qnpool, mpool, ppool = (pools["qk"], pools["qn"], pools["misc"],
                                    pools["p"])
    spool, tpool, opool = pools["st"], pools["t"], pools["o"]
    wq_sb, wk_sb, wvo_sb = consts["wq"], consts["wk"], consts["wvo"]
    tri_sb, cos_sb, sin_sb = consts["tri"], consts["cos"], consts["sin"]

    # rope'd Q^T, K^T: 4 partition-tiles each, [128, S] fp16
    QT = [qkpool.tile([128, S], F16, name=f"b{b}QT{i}", tag=f"QT{i}")
          for i in range(4)]
    KT = [qkpool.tile([128, S], F16, name=f"b{b}KT{i}", tag=f"KT{i}")
          for i in range(4)]
    QN = {}
    # per-(t, chunk) partial row sums, fp32
    rsp = mpool.tile([128, 4 * NT], F32, name=f"b{b}rsp", tag="rsp")
    rsum = mpool.tile([128, NT], F32, name=f"b{b}rsum", tag="rsum")
    rinv = mpool.tile([128, NT], F32, name=f"b{b}rinv", tag="rinv")
    P = {}
    prefetched = {"q": None}

    def proj_units(j, psA, qs_qn=None):
        """Projections + rope for chunk j -> emission units."""
        sl = slice(512 * j, 512 * (j + 1))
        qs, qn = qs_qn if qs_qn is not None else fetch_q(b, j)
        for st in range(4):
            QN[4 * j + st] = qn[st]

        units = []
        for nm, wsb, dst in (("q", wq_sb, QT), ("k", wk_sb, KT)):
            for i in range(2):  # pair-half index
                def u(nm=nm, wsb=wsb, dst=dst, i=i):
                    if deferred_loads is not None and nm == "k" \
                            and i == 0 and j == 0:
                        deferred_loads(0)
                    pe = psA.tile([128, 512], F32,
                                  name=f"b{b}{nm}pe{i}_{j}", tag="pe",
                                  space="PSUM")
                    po = psA.tile([128, 512], F32,
                                  name=f"b{b}{nm}po{i}_{j}", tag="po",
                                  space="PSUM")
                    for zt in range(4):
                        nc.tensor.matmul(
                            pe, wsb[zt][:, 128 * i : 128 * (i + 1)],
                            qs[zt], start=(zt == 0), stop=(zt == 3))
                    for zt in range(4):
                        nc.tensor.matmul(
                            po, wsb[zt][:, 128 * (i + 2) : 128 * (i + 3)],
                            qs[zt], start=(zt == 0), stop=(zt == 3))
                    pe16 = tpool.tile([128, 512], F16,
                                      name=f"pe16_{b}{nm}{i}{j}", tag="pe16")
                    po16 = tpool.tile([128, 512], F16,
                                      name=f"po16_{b}{nm}{i}{j}", tag="po16")
                    nc.scalar.copy(pe16, pe)
                    nc.scalar.copy(po16, po)
                    t1 = tpool.tile([128, 512], F16,
                                    name=f"t1_{b}{nm}{i}{j}", tag="t1")
                    t2 = tpool.tile([128, 512], F16,
                                    name=f"t2_{b}{nm}{i}{j}", tag="t2")
                    t3 = tpool.tile([128, 512], F16,
                                    name=f"t3_{b}{nm}{i}{j}", tag="t3")
                    t4 = tpool.tile([128, 512], F16,
                                    name=f"t4_{b}{nm}{i}{j}", tag="t4")
                    nc.vector.tensor_mul(t1, pe16, cos_sb[i][:, sl])
                    nc.vector.tensor_mul(t2, po16, sin_sb[i][:, sl])
                    nc.gpsimd.tensor_sub(dst[i][:, sl], t1, t2)
                    nc.vector.tensor_mul(t3, pe16, sin_sb[i][:, sl])
                    nc.vector.tensor_mul(t4, po16, cos_sb[i][:, sl])
                    nc.gpsimd.tensor_add(dst[i + 2][:, sl], t3, t4)
                units.append(u)

        def after_qk():
            if deferred_loads is not None and j == 0:
                deferred_loads(1)

        units.append(after_qk)
        return units

    def score_units(j, psS):
        """Score row-tiles t = 4j..4j+3 -> one unit per 512-chunk."""
        units = []
        for t in range(4 * j, 4 * j + 4):
            Kt = 128 * (t + 1)
            nch = j + 1
            p_t = ppool.tile([128, Kt], F16, name=f"b{b}p{t}", tag=f"p{t}")
            P[t] = p_t
            for c in range(nch):
                def uc(t=t, c=c, Kt=Kt, nch=nch, p_t=p_t):
                    w = min(512, Kt - 512 * c)
                    ps = psS.tile([128, 512], F32, name=f"b{b}ps{t}_{c}",
                                  tag="s", space="PSUM")
                    last = c == nch - 1
                    for dt_ in range(4):
                        nc.tensor.matmul(
                            ps[:, :w],
                            QT[dt_][:, 128 * t : 128 * (t + 1)],
                            KT[dt_][:, 512 * c : 512 * c + w],
                            start=(dt_ == 0), stop=(dt_ == 3))
                    psl = p_t[:, 512 * c : 512 * c + w]
                    slot = rsp[:, 4 * t + c : 4 * t + c + 1]
                    if not last:
                        nc.scalar.activation(psl, ps[:, :w], AF.Exp,
                                             accum_out=slot)
                    else:
                        # diagonal block: exp, then 0/1-triangle mask and
                        # row-sum on the DVE (no PE mask matmul)
                        nc.scalar.activation(psl, ps[:, :w], AF.Exp)
                        nc.vector.tensor_mul(psl, psl,
                                             tri_sb[:, 512 - w : 512])
                        nc.vector.tensor_reduce(
                            slot, psl, mybir.AxisListType.X, ALU.add)
                        if nch == 1:
                            nc.vector.reciprocal(rinv[:, t : t + 1], slot)
                        else:
                            nc.vector.tensor_reduce(
                                rsum[:, t : t + 1],
                                rsp[:, 4 * t : 4 * t + nch],
                                mybir.AxisListType.X, ALU.add)
                            nc.vector.reciprocal(rinv[:, t : t + 1],
                                                 rsum[:, t : t + 1])
                        # fold softmax denominator into the qn rows
                        # (contraction index of q^T P)
                        nc.vector.tensor_scalar_mul(
                            QN[t], QN[t], rinv[:, t : t + 1])
                units.append(uc)
        return units

    def qp_pass(j, dts, order, psQP):
        """One 2-bank pass of M = q^T P for output chunk j over dts."""
        po = {dt_: psQP.tile([128, 512], F32,
                             name=f"b{b}po{j}_{dt_}", tag=f"o{dt_ % 2}",
                             space="PSUM")
              for dt_ in dts}
        units = []
        for t in order:
            def ut(t=t):
                n = min(512, 128 * (t + 1) - 512 * j)
                for dt_ in dts:
                    nc.tensor.matmul(
                        po[dt_][:, :n],
                        QN[t][:, 128 * dt_ : 128 * (dt_ + 1)],
                        P[t][:, 512 * j : 512 * j + n],
                        start=(t == order[0]), stop=(t == order[-1]))
            units.append(ut)

        def drain(oT):
            for dt_ in dts:
                o_ = opool.tile([128, 512], F16, name=f"b{b}oT{j}_{dt_}",
                                tag=f"oT{dt_}")
                nc.scalar.copy(o_, po[dt_])
                oT[dt_] = o_
        return units, drain

    def wo_units(j, oT, psWo):
        units = []
        for dot in range(4):
            def ud(dot=dot):
                pf = psWo.tile([128, 512], F32, name=f"b{b}pf{j}_{dot}",
                               tag="f", space="PSUM")
                for dit in range(4):
                    nc.tensor.matmul(
                        pf, wvo_sb[dit][:, 128 * dot : 128 * (dot + 1)],
                        oT[dit], start=(dit == 0), stop=(dit == 3))
                of = opool.tile([128, 512], F32, name=f"b{b}of{j}_{dot}",
                                tag="of")
                nc.scalar.copy(of, pf)
                nc.sync.dma_start(
                    out=outT_d[b, 128 * dot : 128 * (dot + 1),
                               512 * j : 512 * (j + 1)],
                    in_=of)
            units.append(ud)
        return units

    with tc.tile_pool(name=f"psS{b}", bufs=2, space="PSUM") as psS:
        # ---- pipelined: projections(j) interleaved with scores(j-1) ------
        with tc.tile_pool(name=f"psA{b}", bufs=3, space="PSUM") as psA:
            for j in range(4):
                pu = proj_units(j, psA, q0 if j == 0 else None)
                su = score_units(j - 1, psS) if j > 0 else []
                for u in _interleave(pu, su):
                    u()
        # ---- tail: scores(3) overlap q^T P; 2-bank passes + W_vo ---------
        with (
            tc.tile_pool(name=f"psQP{b}", bufs=2, space="PSUM") as psQP,
            tc.tile_pool(name=f"psWo{b}", bufs=2, space="PSUM") as psWo,
        ):
            su = score_units(3, psS)
            # chunk 0: first matmul must cover the whole bank, so start
            # with the earliest full-width tile (t=3); finish with
            # t=12..15 which become ready as the score wave completes
            order0 = [3] + list(range(4, 12)) + [2, 1, 0] + \
                list(range(12, NT))
            orders = {0: order0}
            for j in range(1, 4):
                orders[j] = [4 * j + 3] + list(range(4 * j + 4, NT)) + [
                    4 * j + 2, 4 * j + 1, 4 * j]

            oT = [{} for _ in range(4)]
            pA0, drA0 = qp_pass(0, (0, 1), order0, psQP)
            for u in _interleave(su, pA0[:12]):
                u()
            for u in pA0[12:]:
                u()
            drA0(oT[0])
            pB0, drB0 = qp_pass(0, (2, 3), order0, psQP)
            for u in pB0:
                u()
            drB0(oT[0])
            if prefetch_b is not None:
                # safe now: qp0 (the only reader of QN[0..3] / qs bufs) done
                prefetched["q"] = fetch_q(prefetch_b, 0)
            wo_prev = wo_units(0, oT[0], psWo)
            for j in range(1, 4):
                pA, drA = qp_pass(j, (0, 1), orders[j], psQP)
                pB, drB = qp_pass(j, (2, 3), orders[j], psQP)
                for u in _interleave(pA, wo_prev[:2]):
                    u()
                drA(oT[j])
                for u in _interleave(pB, wo_prev[2:]):
                    u()
                drB(oT[j])
                wo_prev = wo_units(j, oT[j], psWo)
            for u in wo_prev:
                u()
    return prefetched["q"]


def _host_inputs(q, W_q, W_k, W_v, W_o):
    """Build the 8 per-core input maps."""
    scale = 1.0 / math.sqrt(D)
    perm = np.concatenate([np.arange(0, D, 2), np.arange(1, D, 2)])

    q2 = q.reshape(B * S, D)
    qT = np.ascontiguousarray(q2.T).astype(np.float16)
    qn = np.ascontiguousarray(q2).astype(np.float16)

    # trig tables, float32 pipeline mirroring the reference's jnp math
    inv_freq = (1.0 / (10000.0 ** (np.arange(0, D, 2, dtype=np.float32) /
                                   np.float32(D)))).astype(np.float32)
    ang = (np.arange(S, dtype=np.float32)[:, None] * inv_freq[None, :])
    cos2 = np.ascontiguousarray(np.cos(ang, dtype=np.float32).T).astype(
        np.float16)
    sin2 = np.ascontiguousarray(np.sin(ang, dtype=np.float32).T).astype(
        np.float16)

    # [ones(384) | lower-triangle] 0/1 mask; slicing [:, 512-w:] fits any
    # chunk width w with the diagonal 128-block in the last 128 columns
    r = np.arange(128)[:, None]
    c = np.arange(128)[None, :]
    tri01 = np.concatenate(
        [np.ones((128, 384), np.float16),
         (c <= r).astype(np.float16)], axis=1)

    in_maps = []
    for h in range(NCORES):
        wvo = W_v[h].astype(np.float32) @ W_o[D * h : D * (h + 1), :].astype(
            np.float32)
        in_maps.append({
            "qT": qT,
            "qn": qn,
            "wq": np.ascontiguousarray((W_q[h] * scale)[:, perm]).astype(
                np.float16),
            "wk": np.ascontiguousarray(W_k[h][:, perm]).astype(np.float16),
            "wvo": np.ascontiguousarray(wvo).astype(np.float16),
            "cos2": cos2,
            "sin2": sin2,
            "tri01": tri01,
        })
    return in_maps


def kernel(q, W_q, W_k, W_v, W_o):
    from concourse.bass_utils import run_bass_kernel_spmd

    global _BUILT
    q = np.asarray(q, dtype=np.float32)
    W_q = np.asarray(W_q, dtype=np.float32)
    W_k = np.asarray(W_k, dtype=np.float32)
    W_v = np.asarray(W_v, dtype=np.float32)
    W_o = np.asarray(W_o, dtype=np.float32)

    if _BUILT is None:
        _BUILT = build_kernel()
    nc = _BUILT

    in_maps = _host_inputs(q, W_q, W_k, W_v, W_o)
    res = run_bass_kernel_spmd(nc, in_maps, list(range(NCORES)))

    acc = np.zeros((B, S, D), dtype=np.float64)
    for h in range(NCORES):
        acc += res.results[h]["outT"].transpose(0, 2, 1)
    return acc.astype(np.float32)
